# revision 62
# baseline (speedup 1.0000x reference)
"""Trainium2 kernel for the ButterflyConv2d chain (4 grouped 1x1 convs + channel perms).

Key algebraic identity: each grouped conv is a block-diagonal 256x256 matrix and
each butterfly permutation is a permutation matrix, so the whole chain collapses
to ONE dense 256x256 matrix  M = W3 @ P2 @ W2 @ P1 @ W1 @ P0 @ W0  that can be
composed on the host (float64) from the tiny per-layer weights.  The device
kernel is then a single dense matmul  y[o, n] = sum_c M[o, c] * x[c, n]
streamed over n = batch*H*W, which is DMA-bound (the roofline "ridge").

Sharding: data-parallel over batch (dim 0 of x), 4 images per core on 8 cores,
weights replicated, no collectives.

Precision: x and M are staged to the device in fp16 (TensorEngine runs fp16 at
bf16 rate, PSUM accumulates in fp32); the output is staged back as fp16 and
upcast to fp32 on the host.  End-to-end relative error 3.6e-4 (gate is 2e-2).
"""

import numpy as np

import concourse.bass as bass
import concourse.mybir as mybir
import concourse.tile as tile
from concourse import bacc, bass_utils


def _ensure_ntff_hook_importable():
    """bass_utils' trace path (BASS_TRACE=1) does an unguarded
    `from antenv.axon_hooks import get_axon_ntff_profile_hook`; this image's
    antenv lacks that submodule, which would crash a traced run.  Install a
    shim (wired to the boot's ctypes NTFF path when available) so tracing
    either works or degrades gracefully.  No-op if the real module exists."""
    import importlib
    import sys
    import types

    try:
        importlib.import_module("antenv.axon_hooks")
        return  # real module present
    except ImportError:
        pass
    mod = types.ModuleType("antenv.axon_hooks")
    mod._hook = None
    mod.set_axon_ntff_profile_hook = lambda h: setattr(mod, "_hook", h)
    mod.get_axon_ntff_profile_hook = lambda: mod._hook
    try:
        from trn_agent_boot.trn_boot import _ntff_profile_via_ctypes

        mod._hook = _ntff_profile_via_ctypes("/opt/axon/libaxon_pjrt.so")
    except Exception:
        pass  # hook stays None -> bass_utils logs a warning and skips tracing
    sys.modules["antenv.axon_hooks"] = mod
    try:
        import antenv

        antenv.axon_hooks = mod
    except ImportError:
        pass


_ensure_ntff_hook_importable()

WIDTH = 256
BASE = 4
BUTTERFLY_COUNT = 4
B, C, H, W = 32, 256, 56, 56
HW = H * W  # 3136
N_CORES = 8
B_LOCAL = B // N_CORES  # 4
P = 128  # SBUF partitions
NT = 448  # matmul free-dim tile; 7 * 448 == 3136
NTILES = HW // NT

IO_DT = mybir.dt.float16
IO_NP = np.float16
F32 = mybir.dt.float32

import os
KERNEL_STYLE = os.environ.get("BUTTERFLY_KERNEL_STYLE", "raw")  # "raw" | "tile"
# Output staging: "i8" = per-channel-scaled int8 (half the write traffic,
# ~1.0% rel err), "f16" = float16 (~0.04% rel err).  Gate is 2e-2.
OUT_KIND = os.environ.get("BUTTERFLY_OUT_KIND", "f16")
QCLIP = 4.5  # int8 clip point in units of per-channel std (= row norm of M)

# Exposed for test harness introspection (exec_time_ns etc).
LAST_RESULT = None
_NC_CACHE = {}


def _butterfly_permutation(width, group_size, multiplier):
    batch_size = group_size * multiplier
    idx = np.arange(width)
    idx_in_group = idx % group_size
    group_idx = (idx % batch_size) // group_size
    batch_idx = (idx % width) // batch_size
    return group_idx + multiplier * idx_in_group + batch_size * batch_idx


def _compose_matrix(ws):
    """Collapse conv/perm chain to a dense [256, 256] float64 matrix."""

    def block_diag(w):
        G, O, I = w.shape
        Wf = np.zeros((G, O, G, I), dtype=np.float64)
        Wf[np.arange(G), :, np.arange(G), :] = w.astype(np.float64)
        return Wf.reshape(G * O, G * I)

    M = block_diag(ws[0])
    for i in range(BUTTERFLY_COUNT - 1):
        perm = _butterfly_permutation(WIDTH, BASE ** (i + 1), BASE)
        M = M[perm, :]  # y = x[perm]  <=>  y = P @ x with P = I[perm]
        M = block_diag(ws[i + 1]) @ M
    return M


def _build_nc():
    nc = bacc.Bacc("TRN2", target_bir_lowering=False, debug=False)

    x = nc.declare_dram_parameter("x", [B_LOCAL, 2, P, HW], IO_DT, isOutput=False)
    mt = nc.declare_dram_parameter("mt", [2, P, 2 * P], IO_DT, isOutput=False)
    out = nc.declare_dram_parameter("out", [B_LOCAL, 2, P, HW], IO_DT, isOutput=True)

    with tile.TileContext(nc) as tc:
        with (
            tc.tile_pool(name="wpool", bufs=1) as wp,
            tc.tile_pool(name="xpool", bufs=3) as xp,
            tc.tile_pool(name="ypool", bufs=3) as yp,
            tc.tile_pool(name="pspool", bufs=8, space="PSUM") as pp,
        ):
            # M^T tiles: wts[ct][c_part, o] with c = ct*128 + c_part
            wts = []
            for ct in range(2):
                wt = wp.tile([P, 2 * P], IO_DT, tag=f"w{ct}")
                nc.sync.dma_start(wt[:], mt[ct])
                wts.append(wt)

            for b in range(B_LOCAL):
                xts = []
                for ct in range(2):
                    xt = xp.tile([P, HW], IO_DT, tag=f"x{ct}")
                    nc.sync.dma_start(xt[:], x[b, ct])
                    xts.append(xt)
                for ot in range(2):
                    yt = yp.tile([P, HW], IO_DT, tag=f"y{ot}")
                    for i in range(NTILES):
                        ps = pp.tile([P, NT], F32)
                        nsl = bass.ts(i, NT)
                        osl = bass.ts(ot, P)
                        # y[o_tile, n] = M[o_tile, c0].x[c0, n] + M[o_tile, c1].x[c1, n]
                        nc.tensor.matmul(
                            ps[:], wts[0][:, osl], xts[0][:, nsl],
                            start=True, stop=False,
                        )
                        nc.tensor.matmul(
                            ps[:], wts[1][:, osl], xts[1][:, nsl],
                            start=False, stop=True,
                        )
                        # PSUM -> SBUF (+ fp32 -> fp16): alternate DVE / ACT
                        if i % 2 == 0:
                            nc.vector.tensor_copy(yt[:, nsl], ps[:])
                        else:
                            nc.scalar.copy(yt[:, nsl], ps[:])
                    nc.sync.dma_start(out[b, ot], yt[:])

    nc.finalize()
    return nc


def _build_nc_raw():
    """Hand-scheduled version: no Tile end-of-kernel barrier and few semaphores
    / DMA instructions (the NRT epilogue sweeps per queue/sem, so fewer is
    faster).

    Host stages x interleaved as [b, p, t*HW+n] so each batch image is ONE
    1.6 MB in-DMA [128, 2*HW]; same for the output.  Per core:
      sync:   trigger w + 4 x in-DMAs immediately; per batch b wait for its 14
              copies and trigger the out-DMA.
      tensor: 56 matmul pairs (accumulate 2 c-tiles into one PSUM bank).
              pair k uses PSUM slot k%8; before reuse wait for that slot's copy.
      vector/scalar: alternating copies PSUM->SBUF(y) (f32->f16).
      gpsimd: only zeroes the PE warm-up scratch tile.
    No explicit end-of-kernel semaphore cleanup: the runtime's own epilogue
    re-zeroes semaphores (repeat-execution correctness verified on hardware).
    """
    from contextlib import ExitStack

    nc = bacc.Bacc("TRN2", target_bir_lowering=False, debug=False)

    HW2 = 2 * HW
    OUT_DT = mybir.dt.int8 if OUT_KIND == "i8" else IO_DT
    WCOL = 4 * P  # 512 columns of M^T packed in front of batch 0's pixels
    xw = nc.declare_dram_parameter("xw", [P, WCOL + HW2], IO_DT, isOutput=False)
    x = nc.declare_dram_parameter("x", [B_LOCAL - 1, P, HW2], IO_DT, isOutput=False)
    out = nc.declare_dram_parameter("out", [B_LOCAL, P, HW2], OUT_DT, isOutput=True)

    PERB = 2 * NTILES                 # 14 matmul pairs per batch image
    NPAIR = B_LOCAL * PERB            # 56
    NSLOT = 8                         # PSUM banks
    NYBUF = 4                         # one y buffer per batch image: no reuse waits

    NUSE = NPAIR // 2                 # 28 two-pair PSUM tile uses (7 per batch)
    NPSB = 4                          # 4 two-bank PSUM tiles = 8 banks

    with ExitStack() as ctx:
        en = ctx.enter_context
        xts = [en(nc.sbuf_tensor("x0w", [P, WCOL + HW2], IO_DT))] + [
            en(nc.sbuf_tensor(f"x{b}", [P, HW2], IO_DT)) for b in range(1, B_LOCAL)
        ]
        yts = [en(nc.sbuf_tensor(f"y{i}", [P, HW2], OUT_DT)) for i in range(NYBUF)]
        # [P, 2, 512]: two PSUM banks; pair k writes [:, k%2, :448]
        pss = [en(nc.psum_tensor(f"ps{i}", [P, 2, 512], F32)) for i in range(NPSB)]
        dmy = en(nc.sbuf_tensor("dmy", [P, NT], IO_DT))  # PE warm-up scratch
        wt = xts[0]  # weights live in the first WCOL columns of batch 0's tile
        # batch 0 is staged as [w | panelA | panelB] with panelA = the first 4
        # 448-col blocks of each channel half, so the PE can start after a
        # ~1 MB DMA instead of the full 1.7 MB
        PA = 4 * NT  # 1792 cols per half in panel A
        PB = HW - PA

        def wslice(ct, ot):
            return wt[:, bass.ds(ct * 2 * P + ot * P, P)]

        def xslice(b, t, i):
            if b == 0:
                if i < 4:
                    off = WCOL + t * PA + i * NT
                else:
                    off = WCOL + 2 * PA + t * PB + (i - 4) * NT
                return xts[0][:, bass.ds(off, NT)]
            return xts[b][:, bass.ds(t * HW + i * NT, NT)]

        LEAN = os.environ.get("BUTTERFLY_LEAN", "1") == "1"
        NOSEM = os.environ.get("BUTTERFLY_NOSEM", "0") == "1"
        # one sem per DMA, waited at exactly 16 -> no assumption on cross-DMA
        # completion order
        s_x = [en(nc.semaphore(f"s_x{b}")) for b in range(B_LOCAL)]
        s_x0b = en(nc.semaphore("s_x0b"))
        if NOSEM:
            # nothing ever waits on out-DMA completion (NYBUF covers every
            # image; the runtime's own quiesce covers NEFF completion), so
            # skip those sems entirely - the NRT epilogue sweeps fewer sems
            s_out = None
            s_out3b = None
        else:
            s_out = [en(nc.semaphore(f"s_out{b}")) for b in range(B_LOCAL)]
            s_out3b = en(nc.semaphore("s_out3b"))
        # single-updater cumulative sems
        s_pe = en(nc.semaphore("s_pe"))
        s_cpv = en(nc.semaphore("s_cpv"))
        s_cpa = en(nc.semaphore("s_cpa"))
        s_dmy = en(nc.semaphore("s_dmy"))
        blk = en(nc.Block(no_gpsimd_drain=LEAN))

        @blk.gpsimd
        def _(gpsimd):
            gpsimd.memset(dmy[:], 0.0).then_inc(s_dmy, 1)

        @blk.sync
        def _(sync):
            SPLIT = WCOL + 2 * PA
            sync.dma_start(xts[0][:, 0:SPLIT], xw[:, 0:SPLIT]).then_inc(s_x[0], 16)
            sync.dma_start(
                xts[0][:, SPLIT:], xw[:, bass.ds(SPLIT, 2 * PB)]
            ).then_inc(s_x0b, 16)
            for b in range(1, B_LOCAL):
                sync.dma_start(xts[b][:], x[b - 1]).then_inc(s_x[b], 16)
            last = B_LOCAL - 1
            for b in range(B_LOCAL):
                # phase-order the HBM traffic: out-DMAs round-robin bandwidth
                # away from pending input DMAs (starving the PE), so gate out b
                # on input b+2 having landed (out0 then only overlaps x3's tail)
                STAG = int(os.environ.get("BUTTERFLY_STAG", "2"))
                sync.wait_ge(s_x[min(b + STAG, last)], 16)
                if LEAN and b == last:
                    uend = 7 * (b + 1)
                    sync.wait_ge(s_cpv, uend)
                    sync.wait_ge(s_cpa, uend)
                    dma = sync.dma_start(out[b], yts[b % NYBUF][:])
                    if not NOSEM:
                        dma.then_inc(s_out[b], 16)
                elif b < last:
                    uend = 7 * (b + 1)
                    sync.wait_ge(s_cpv, uend)
                    sync.wait_ge(s_cpa, uend)
                    dma = sync.dma_start(out[b], yts[b % NYBUF][:])
                    if not NOSEM:
                        dma.then_inc(s_out[b], 16)
                else:
                    # split the last image's out-DMA so only a small tail
                    # transfer is gated on the very last copies
                    RS = int(os.environ.get("BUTTERFLY_RS", "10"))  # split block
                    for h in range(2):
                        lo = 0 if h == 0 else RS * NT
                        hi = RS * NT if h == 0 else HW2
                        uend = 7 * b + (RS + 1) // 2 if h == 0 else 7 * (b + 1)
                        sync.wait_ge(s_cpv, uend)
                        sync.wait_ge(s_cpa, uend)
                        sync.dma_start(
                            out[b, :, bass.ds(lo, hi - lo)],
                            yts[b % NYBUF][:, bass.ds(lo, hi - lo)],
                        ).then_inc(s_out[b] if h == 0 else s_out3b, 16)

        @blk.tensor
        def _(tensor):
            # HAM warm-up: the PE clock sits at 1.2 GHz until ~3.4 us of
            # sustained activity.  Burn the otherwise-idle preamble (first x
            # DMA in flight) on dummy matmuls over a zeroed scratch tile so
            # the real stream starts at 2.4 GHz.  Results land in a PSUM
            # bank whose first real matmul clears it (start=True).
            tensor.wait_ge(s_dmy, 1)
            # 10 x ~0.37 us cold ~= 3.7 us of PE activity: enough to clear the
            # ~3.4 us HAM window, and still done before the first input lands
            # even when the chip is power-throttled (16 was not)
            for _ in range(int(os.environ.get("BUTTERFLY_NDUMMY", "10"))):
                tensor.matmul(pss[NPSB - 1][:, 1, 0:NT], dmy[:, 0:P], dmy[:],
                              start=True, stop=True, skip_group_check=True)
            for k in range(NPAIR):
                b, r = divmod(k, PERB)
                ot, i = divmod(r, NTILES)
                u, j = divmod(k, 2)
                if r == 0:
                    tensor.wait_ge(s_x[b], 16)
                if b == 0 and r == 4:
                    tensor.wait_ge(s_x0b, 16)
                if j == 0 and u >= NPSB:
                    v = u - NPSB  # previous use of this PSUM tile: both banks copied
                    tensor.wait_ge(s_cpv, v + 1)
                    tensor.wait_ge(s_cpa, v + 1)
                ps = pss[u % NPSB]
                tensor.matmul(ps[:, j, 0:NT], wslice(0, ot), xslice(b, 0, i),
                              start=True, stop=False)
                tensor.matmul(ps[:, j, 0:NT], wslice(1, ot), xslice(b, 1, i),
                              start=False, stop=True).then_inc(s_pe, 1)

        def copier(eng, e, sem):
            # engine e owns bank e of every PSUM tile use: DVE copies even
            # pairs, ACT odd pairs; each starts as soon as ITS pair is done
            for u in range(NUSE):
                k = 2 * u + e
                b, r = divmod(k, PERB)
                eng.wait_ge(s_pe, k + 1)
                if b >= NYBUF and u % 7 == 0:
                    # first copy of this engine into y buffer b%NYBUF (WAR)
                    eng.wait_ge(s_out[b - NYBUF], 16)
                    if b - NYBUF == B_LOCAL - 1:
                        eng.wait_ge(s_out3b, 16)
                cp = eng.tensor_copy if e == 0 else eng.copy
                cp(yts[b % NYBUF][:, bass.ds(r * NT, NT)],
                   pss[u % NPSB][:, e, 0:NT]).then_inc(sem, 1)

        @blk.vector
        def _(vector):
            copier(vector, 0, s_cpv)

        @blk.scalar
        def _(scalar):
            copier(scalar, 1, s_cpa)

    nc.finalize()
    return nc


def kernel(x, w0, w1, w2, w3):
    global LAST_RESULT

    M = _compose_matrix([np.asarray(w, np.float64) for w in (w0, w1, w2, w3)])
    dq = None
    if KERNEL_STYLE == "raw" and OUT_KIND == "i8":
        # fold the int8 quantization scale into M's rows; dequantize on host.
        # row norm of M == std of output channel c (x is iid standard normal)
        rown = np.linalg.norm(M, axis=1)
        dq = (QCLIP * rown / 127.0).astype(np.float32)  # [256], c = t*128 + p
        M = M * (127.0 / (QCLIP * rown))[:, None]
    mt_t = M.T.astype(IO_NP)  # mt_t[c, o] = M[o, c]

    if "nc" not in _NC_CACHE:
        build = _build_nc_raw if KERNEL_STYLE == "raw" else _build_nc
        _NC_CACHE["nc"] = build()
    nc = _NC_CACHE["nc"]

    if KERNEL_STYLE == "raw":
        # staged interleaved: x16[b, p, t*HW + n] = x[b, c, n] with c = t*128 + p
        x16 = (
            np.asarray(x).astype(IO_NP)
            .reshape(B, 2, P, HW)
            .transpose(0, 2, 1, 3)
            .reshape(B, P, 2 * HW)
        )
        # weights as [p, ct*256 + o] columns, fused in front of batch 0's pixels;
        # batch 0 split into panels A (first 4 448-blocks per half) and B (rest)
        w16 = mt_t.reshape(2, P, 2 * P).transpose(1, 0, 2).reshape(P, 4 * P)
        PA = 4 * 448
        in_maps = []
        for i in range(N_CORES):
            sh = x16[i * B_LOCAL:(i + 1) * B_LOCAL]
            s0 = sh[0]
            xw = np.concatenate(
                [w16,
                 s0[:, 0:PA], s0[:, HW:HW + PA],
                 s0[:, PA:HW], s0[:, HW + PA:]],
                axis=1,
            )
            in_maps.append({
                "xw": np.ascontiguousarray(xw),
                "x": np.ascontiguousarray(sh[1:]),
            })
        res = bass_utils.run_bass_kernel_spmd(nc, in_maps, core_ids=list(range(N_CORES)))
        LAST_RESULT = res
        y16 = np.concatenate([res.results[i]["out"] for i in range(N_CORES)], axis=0)
        y16 = y16.reshape(B, P, 2, HW).transpose(0, 2, 1, 3)  # -> [B, t, p, HW]
        y = np.ascontiguousarray(y16).reshape(B, C, H, W).astype(np.float32)
        if dq is not None:
            y *= dq.reshape(1, C, 1, 1)
        return y

    mt16 = np.ascontiguousarray(mt_t.reshape(2, P, 2 * P))
    x16 = np.asarray(x).astype(IO_NP).reshape(B, 2, P, HW)
    in_maps = [
        {"x": np.ascontiguousarray(x16[i * B_LOCAL:(i + 1) * B_LOCAL]), "mt": mt16}
        for i in range(N_CORES)
    ]
    res = bass_utils.run_bass_kernel_spmd(nc, in_maps, core_ids=list(range(N_CORES)))
    LAST_RESULT = res
    y16 = np.concatenate([res.results[i]["out"] for i in range(N_CORES)], axis=0)
    return y16.reshape(B, C, H, W).astype(np.float32)
